# revision 49
# baseline (speedup 1.0000x reference)
"""Block-sparse MoE (sparse expert-parallel dispatch) Trainium2 kernel.

Problem: nn_BlockSparseMoE_15882789061249
  T=1024 tokens, H=2048 hidden, F=1408 intermediate, E=16 experts, top_k=6.

Strategy (8 NeuronCores, SPMD single program):
  - Expert parallel: core c owns experts {2c, 2c+1}; wv1/w2 sharded by
    expert on the host, gate replicated (columns permuted per core so the
    own experts land in route columns 0/1 -> one SPMD program).
  - Sparse dispatch: only top_k=6 of 16 experts contribute per token, so
    each expert needs only ~6/16 of the tokens. The host computes the
    routing *metadata* (which tokens each expert needs, with a 1e-4
    relative margin around the 6th prob so host/device top-k can never
    disagree) and ships per-expert gathered token matrices of capacity
    C=512 (actual max count is 418). All *numerics* stay on device: the
    fp32 router (logits -> softmax -> top-6 -> renorm), the expert MLPs
    on the gathered tokens, the route-weight combine, and the cross-core
    reduce-scatter.
  - Slots are bucketed by token-tile *pair* (4 buckets x 128 slots per
    expert; max actual bucket is 112), which makes the scatter-back
    pattern compile-time static: slot-chunk j only touches token tiles
    2j/2j+1. Scatter-back is a matmul with a host-provided 0/1 selection
    matrix, weighted on-device by the routed probabilities.
  - Weights are laid out so every DMA line is 2-4 KiB contiguous (the
    old per-[128,128]-tile layout moved 256 B lines and throttled the
    PE array to ~60% in phase A).
  - The reduce-scatter runs in 4 chunks of 2 token tiles, each fired as
    soon as its partial is complete, hiding most of the collective
    behind compute. Each core emits 4x [32, 2048] shards; the host
    reassembles them.
"""

import numpy as np

T, H, F, E = 1024, 2048, 1408, 16
NCORES = 8
TOPK = 6
EPC = E // NCORES            # experts per core (2)
NB = 4                       # slot buckets per expert (token-tile pairs)
KH = H // 128                # 16
KF = F // 128                # 11
MF2 = 2 * F // 128           # 22
MT = T // 128                # 8 token tiles
MARGIN = 1e-4                # relative margin on the 6th prob

_CACHE = {}


def build_moe_nc(n_cores=NCORES, BK=112):
    """Build + compile the SPMD Bass program for one core (same for all).

    BK = slot-bucket capacity (max tokens any expert draws from one
    token-tile pair, rounded up to 8). C = NB*BK is the per-expert
    gathered-token capacity; smaller BK means proportionally less
    phase-A matmul time, so it is fitted to the actual routing.
    """
    import concourse.bacc as bacc
    import concourse.mybir as mybir
    import concourse.tile as tile

    C = NB * BK

    f32 = mybir.dt.float32
    bf16 = mybir.dt.bfloat16
    AF = mybir.ActivationFunctionType
    Alu = mybir.AluOpType
    X = mybir.AxisListType.X

    t, e = T, E
    nc = bacc.Bacc("TRN2", target_bir_lowering=False, debug=False,
                   num_devices=n_cores)

    xT = nc.dram_tensor("xT", [H, t], f32, kind="ExternalInput")
    gwp = nc.dram_tensor("gwp", [128, KH * e], f32, kind="ExternalInput")
    xgd = nc.dram_tensor("xgd", [KH, 128, EPC * C], bf16,
                         kind="ExternalInput")
    seld = nc.dram_tensor("seld", [EPC, NB, BK, t], bf16,
                          kind="ExternalInput")
    wgd = nc.dram_tensor("wgd", [EPC, MF2, 128, KH * 128], bf16,
                         kind="ExternalInput")
    w2d = nc.dram_tensor("w2d", [EPC, KF, 128, H], bf16,
                         kind="ExternalInput")

    shw = 2 * 128 // n_cores
    parts = [nc.dram_tensor(f"partial{j}", [2 * 128, H], bf16)
             for j in range(NB)]
    rss = [nc.dram_tensor(f"rsi{j}", [shw, H], bf16) for j in range(NB)]
    out_sh = nc.dram_tensor("out_sh", [NB * shw, H], bf16,
                            kind="ExternalOutput")
    wrm_i = nc.dram_tensor("wrm_i", [8, 256], bf16)
    wrm_o = nc.dram_tensor("wrm_o", [1, 256], bf16)
    wrm2_i = nc.dram_tensor("wrm2_i", [8, 256], bf16)
    wrm2_o = nc.dram_tensor("wrm2_o", [1, 256], bf16)

    W2PRE = 8                # e0 w2 k-tiles prefetched before phase B

    with tile.TileContext(nc) as tc:
        with tc.tile_pool(name="persist", bufs=1) as pp:
            gw = pp.tile([128, KH * e], f32, tag="gw")
            lg = pp.tile([128, t], f32, tag="lg")
            route = pp.tile([128, MT * e + 32], f32, tag="route")
            ltr = pp.tile([128, MT * 32], f32, tag="ltr")
            rqs = pp.tile([128, EPC * t], f32, tag="rqs")
            rbc = pp.tile([128, EPC * t], f32, tag="rbc")
            act = pp.tile([128, EPC * KF * C], bf16, tag="act")
            sels = pp.tile([128, EPC * NB * t], bf16, tag="sels")
            selw = pp.tile([128, EPC * NB * t], bf16, tag="selw")
            w2p0 = pp.tile([128, W2PRE * H], bf16, tag="w2p0")
            ones = pp.tile([128, 128], f32, tag="ones")

            nc.sync.dma_start(out=gw[:], in_=gwp[:, :])
            nc.vector.memset(ones[0:32, :], 1.0)

            # rows 16:32 of lg feed the padded 32x32 transposes below; the
            # copy from psl overwrites rows :16 afterwards (32-aligned base)
            nc.vector.memset(lg[0:32, :], 0.0)
            nc.vector.memset(route[:, MT * e:], 0.0)
            nc.vector.memset(rqs[0:32, :], 0.0)

            # tiny collective up front absorbs the cold-start cost of the
            # CC path so the first real reduce-scatter runs at ring speed
            nc.gpsimd.collective_compute(
                "ReduceScatter", Alu.add,
                replica_groups=[list(range(n_cores))],
                ins=[wrm_i.ap().opt()],
                outs=[wrm_o.ap().opt()],
            )

            with (tc.tile_pool(name="xg", bufs=1) as pxg,
                  tc.tile_pool(name="wv", bufs=4) as pwv,
                  tc.tile_pool(name="xf", bufs=3) as pxf,
                  tc.tile_pool(name="sg", bufs=3) as psg,
                  tc.tile_pool(name="rt", bufs=2) as prt,
                  tc.tile_pool(name="psa", bufs=2, space="PSUM") as ppa):
                xg = pxg.tile([128, KH * EPC * C], bf16, tag="xg")

                def xg_dma(k):
                    nc.sync.dma_start(
                        out=xg[:, k * EPC * C:(k + 1) * EPC * C],
                        in_=xgd[k])

                # first few expert-pair weights and the xg tiles pair 0
                # consumes immediately go ahead of everything else in the
                # DMA queues so the PE can start within a few us
                NPRE = 3
                wpre = {}
                for mm in range(NPRE):
                    wg = pwv.tile([128, KH * 128], bf16, tag="wg",
                                  name=f"wgp{mm}")
                    nc.sync.dma_start(out=wg[:], in_=wgd[0, mm])
                    wu = pwv.tile([128, KH * 128], bf16, tag="wu",
                                  name=f"wup{mm}")
                    nc.sync.dma_start(out=wu[:], in_=wgd[0, KF + mm])
                    wpre[(0, mm)] = (wg, wu)
                    if mm == 0:
                        for k in range(4):
                            xg_dma(k)
                for k in range(4, KH):
                    xg_dma(k)

                def emit_a(le, mm, wgt, wut):
                    pg = ppa.tile([128, C], f32, tag="pg", name=f"pg{le}_{mm}")
                    pu = ppa.tile([128, C], f32, tag="pu", name=f"pu{le}_{mm}")
                    for k in range(KH):
                        rh = xg[:, k * EPC * C + le * C:
                                k * EPC * C + (le + 1) * C]
                        nc.tensor.matmul(pg[:],
                                         lhsT=wgt[:, k * 128:(k + 1) * 128],
                                         rhs=rh,
                                         start=(k == 0), stop=(k == KH - 1))
                    for k in range(KH):
                        rh = xg[:, k * EPC * C + le * C:
                                k * EPC * C + (le + 1) * C]
                        nc.tensor.matmul(pu[:],
                                         lhsT=wut[:, k * 128:(k + 1) * 128],
                                         rhs=rh,
                                         start=(k == 0), stop=(k == KH - 1))
                    sgm = psg.tile([128, C], bf16, tag="sgm",
                                   name=f"sgm{le}_{mm}")
                    nc.scalar.activation(sgm[:], pg[:], AF.Sigmoid)
                    sg = psg.tile([128, C], bf16, tag="sg",
                                  name=f"sg{le}_{mm}")
                    nc.vector.tensor_mul(out=sg[:], in0=sgm[:], in1=pg[:])
                    ai = (le * KF + mm) * C
                    nc.vector.tensor_mul(out=act[:, ai:ai + C],
                                         in0=sg[:], in1=pu[:])

                def emit_a_range(pairs):
                    for le, mm in pairs:
                        if (le, mm) in wpre:
                            emit_a(le, mm, *wpre[(le, mm)])
                            continue
                        wg = pwv.tile([128, KH * 128], bf16, tag="wg",
                                      name=f"wg{le}_{mm}")
                        nc.sync.dma_start(out=wg[:], in_=wgd[le, mm])
                        wu = pwv.tile([128, KH * 128], bf16, tag="wu",
                                      name=f"wu{le}_{mm}")
                        nc.sync.dma_start(out=wu[:], in_=wgd[le, KF + mm])
                        emit_a(le, mm, wg, wu)

                all_pairs = [(le, mm) for le in range(EPC)
                             for mm in range(KF)]
                # phase A, first slice: keeps the PE warm while the fp32
                # router matmuls (below) slot into the middle of the stream
                emit_a_range(all_pairs[:6])
                for le in range(EPC):
                    for jj in range(NB):
                        blk = (le * NB + jj) * t
                        nc.sync.dma_start(out=sels[0:BK, blk:blk + t],
                                          in_=seld[le, jj])
                emit_a_range(all_pairs[6:14])

                # ---- router logits [e, t] (fp32) ----
                # M=16 matmuls packed 2-wide into PE column strips: even k
                # into strip 0, odd k into strip 1, streaming concurrently.
                # Each strip owns its PSUM banks with its own start/stop
                # group (strips sharing a bank corrupts has_written state),
                # and xf tiles stay full-width so the DMA pattern is
                # unchanged from the serial version.
                with tc.tile_pool(name="psr", bufs=1,
                                  space="PSUM") as ppr:
                    psls = [ppr.tile([128, t], f32, tag=f"psl{j}",
                                     name=f"psl{j}") for j in range(2)]
                    for k in range(KH):
                        j = k % 2
                        xf = pxf.tile([128, t], f32, tag="xf",
                                      name=f"xf{k}")
                        nc.sync.dma_start(out=xf[:],
                                          in_=xT[k * 128:(k + 1) * 128, :])
                        for n0 in range(0, t, 512):
                            nc.tensor.matmul(
                                psls[j][32 * j:32 * j + e, n0:n0 + 512],
                                lhsT=gw[:, k * e:(k + 1) * e],
                                rhs=xf[:, n0:n0 + 512],
                                start=(k < 2), stop=(k >= KH - 2),
                                tile_position=(0, 32 * j),
                                skip_group_check=True)
                    sA = prt.tile([128, t], f32, tag="sA", bufs=1)
                    nc.vector.tensor_copy(out=sA[0:16, :],
                                          in_=psls[0][0:16, :])
                    nc.vector.tensor_add(lg[0:16, :], sA[0:16, :],
                                         psls[1][32:48, :])

                # ---- top-k per token tile (all DVE; PE stays on MLPs) ----
                for tt in range(MT):
                    for b in range(4):
                        nc.vector.transpose(
                            out=ltr[b * 32:(b + 1) * 32,
                                    tt * 32:(tt + 1) * 32],
                            in_=lg[0:32, tt * 128 + b * 32:
                                   tt * 128 + (b + 1) * 32])
                    ev_in = ltr[:, tt * 32:tt * 32 + e]
                    mx = prt.tile([128, 1], f32, tag="mx")
                    nc.vector.reduce_max(out=mx[:], in_=ev_in, axis=X)
                    nm = prt.tile([128, 1], f32, tag="nm")
                    nc.vector.tensor_scalar_mul(nm[:], mx[:], -1.0)
                    ev = prt.tile([128, e], f32, tag="ev")
                    nc.scalar.activation(ev[:], ev_in, AF.Exp,
                                         bias=nm[:], scale=1.0)
                    t8 = prt.tile([128, 8], f32, tag="t8")
                    nc.vector.max(out=t8[:], in_=ev[:])
                    nc.vector.memset(t8[:, TOPK:], 0.0)
                    zap = prt.tile([128, e], f32, tag="zap")
                    nc.vector.match_replace(out=zap[:], in_to_replace=t8[:],
                                            in_values=ev[:], imm_value=0.0)
                    msk = prt.tile([128, e], f32, tag="msk")
                    nc.vector.tensor_sub(msk[:], ev[:], zap[:])
                    dn = prt.tile([128, 1], f32, tag="dn")
                    nc.vector.reduce_sum(out=dn[:], in_=msk[:], axis=X)
                    iv = prt.tile([128, 1], f32, tag="iv")
                    nc.vector.reciprocal(iv[:], dn[:])
                    nc.vector.tensor_scalar_mul(
                        route[:, tt * e:(tt + 1) * e], msk[:], iv[:])

                # route -> [expert, token] layout. The transpose input is
                # shifted by `le` so own-expert column le lands on partition
                # 0 of its block (compute APs need 32-aligned partition
                # bases, so reading rqs[1:2, :] later would be illegal).
                for le in range(EPC):
                    for tt in range(MT):
                        for b in range(4):
                            nc.vector.transpose(
                                out=rqs[0:32, le * t + tt * 128 + b * 32:
                                        le * t + tt * 128 + (b + 1) * 32],
                                in_=route[b * 32:(b + 1) * 32,
                                          tt * e + le:tt * e + le + 32])
                # ---- phase A, second slice, with the w2 prefetch DMAs
                # interleaved so they land just before phase B needs them
                emit_a_range(all_pairs[14:17])
                for kk in range(W2PRE):
                    nc.sync.dma_start(out=w2p0[:, kk * H:(kk + 1) * H],
                                      in_=w2d[0, kk])
                emit_a_range(all_pairs[17:])

                # broadcast each own-expert route row across all 128
                # partitions via a PE outer product (ones x row). Keeping
                # this off GpSimd matters: gpsimd ucode ops ahead of the
                # collectives were observed to stall the whole CC chain.
                with tc.tile_pool(name="psb2", bufs=1,
                                  space="PSUM") as ppb2:
                    for le in range(EPC):
                        pbc = ppb2.tile([128, t], f32, tag="pbc",
                                        name=f"pbc{le}")
                        for n0 in range(0, t, 512):
                            nc.tensor.matmul(
                                pbc[:, n0:n0 + 512],
                                lhsT=ones[0:1, :],
                                rhs=rqs[0:1, le * t + n0:le * t + n0 + 512],
                                start=True, stop=True)
                        nc.vector.tensor_copy(
                            out=rbc[:, le * t:(le + 1) * t], in_=pbc[:])
                for le in range(EPC):
                    for jj in range(NB):
                        blk = (le * NB + jj) * t
                        nc.vector.tensor_mul(
                            out=selw[0:BK, blk:blk + t],
                            in0=sels[0:BK, blk:blk + t],
                            in1=rbc[0:BK, le * t:(le + 1) * t])

            # second warm-up collective, data-gated on the end of phase A:
            # without it the CC path sits cold for ~160us and the first
            # real reduce-scatter ran 2-3x slower than the rest
            nc.sync.dma_start(out=wrm2_i[:, :],
                              in_=act[0:8, EPC * KF * C - 256:])
            nc.gpsimd.collective_compute(
                "ReduceScatter", Alu.add,
                replica_groups=[list(range(n_cores))],
                ins=[wrm2_i.ap().opt()],
                outs=[wrm2_o.ap().opt()],
            )

            # ---- phase B + weighted scatter + chunked reduce-scatter ----
            with (tc.tile_pool(name="w2p", bufs=EPC * KF - W2PRE) as pw2,
                  tc.tile_pool(name="yb", bufs=4) as pyb,
                  tc.tile_pool(name="so", bufs=2) as pso,
                  tc.tile_pool(name="psb", bufs=3, space="PSUM") as ppb,
                  tc.tile_pool(name="psc", bufs=2, space="PSUM") as ppc):
                w2sb = {}
                for le in range(EPC):
                    for kk in range(KF):
                        if le == 0 and kk < W2PRE:
                            w2sb[(le, kk)] = w2p0[:, kk * H:(kk + 1) * H]
                        else:
                            w2k = pw2.tile([128, H], bf16, tag="w2k")
                            nc.sync.dma_start(out=w2k[:], in_=w2d[le, kk])
                            w2sb[(le, kk)] = w2k[:]

                for jj in range(NB):
                    ybs = {}
                    for le in range(EPC):
                        py = [ppb.tile([128, 1024], f32, tag="py",
                                       name=f"py{jj}_{le}_{hh}")
                              for hh in range(2)]
                        for kk in range(KF):
                            lh = act[:, (le * KF + kk) * C + jj * BK:
                                     (le * KF + kk) * C + (jj + 1) * BK]
                            w2t_ = w2sb[(le, kk)]
                            for hh in range(2):
                                for q in range(2):
                                    n0 = q * 512
                                    nc.tensor.matmul(
                                        py[hh][0:BK, n0:n0 + 512],
                                        lhsT=lh,
                                        rhs=w2t_[:, hh * 1024 + n0:
                                                 hh * 1024 + n0 + 512],
                                        start=(kk == 0), stop=(kk == KF - 1))
                        yb = pyb.tile([128, H], bf16, tag="yb")
                        # ScalarE drains py so the DVE stays free for the
                        # scatter copies that gate the partial DMAs
                        for hh in range(2):
                            nc.scalar.activation(
                                yb[0:BK, hh * 1024:(hh + 1) * 1024],
                                py[hh][0:BK, :], AF.Copy)
                        ybs[le] = yb

                    for tt in (2 * jj, 2 * jj + 1):
                        so = pso.tile([128, H], bf16, tag="so",
                                      name=f"so{tt}")
                        for hq in range(4):
                            ps = ppc.tile([128, 512], f32, tag="ps",
                                          name=f"ps{tt}_{hq}")
                            for le in range(EPC):
                                blk = (le * NB + jj) * t
                                nc.tensor.matmul(
                                    ps[:],
                                    lhsT=selw[0:BK, blk + tt * 128:
                                              blk + (tt + 1) * 128],
                                    rhs=ybs[le][0:BK,
                                                hq * 512:(hq + 1) * 512],
                                    start=(le == 0), stop=(le == EPC - 1))
                            nc.vector.tensor_copy(
                                out=so[:, hq * 512:(hq + 1) * 512],
                                in_=ps[:])
                        # spread each partial store over 4 row-slice DMAs
                        # alternating between both DMA engines' ring sets:
                        # one ring moves ~40 GB/s, and backlogged partials
                        # delayed the reduce-scatter triggers 20-30 us
                        r0 = (tt % 2) * 128
                        for q in range(4):
                            eng = nc.sync if q % 2 == 0 else nc.scalar
                            eng.dma_start(
                                out=parts[jj][r0 + q * 32:
                                              r0 + (q + 1) * 32, :],
                                in_=so[q * 32:(q + 1) * 32, :])

                # collectives are emitted after the compute loop: each is
                # data-gated on its partial, so they still overlap B/scatter
                # of later chunks, but no sync edges land inside the PE/DVE
                # streams (in-loop emission measurably stalled both)
                for jj in range(NB):
                    nc.gpsimd.collective_compute(
                        "ReduceScatter", Alu.add,
                        replica_groups=[list(range(n_cores))],
                        ins=[parts[jj].ap().opt()],
                        outs=[rss[jj].ap().opt()],
                    )
                    nc.sync.dma_start(
                        out=out_sh[jj * shw:(jj + 1) * shw, :],
                        in_=rss[jj][:, :])

    nc.compile()
    return nc


def _route_sel(x, gate_w):
    """Host routing metadata: top-6 membership with a tie margin."""
    lg = x.astype(np.float64) @ gate_w.astype(np.float64).T
    lg -= lg.max(axis=1, keepdims=True)
    p = np.exp(lg)
    p /= p.sum(axis=1, keepdims=True)
    sp = -np.sort(-p, axis=1)
    thr = sp[:, TOPK - 1:TOPK] * (1.0 - MARGIN)
    return p >= thr


def fit_bk(sel):
    """Slot-bucket capacity: max (expert, tile-pair) count, rounded to 8."""
    tp = np.arange(T) // (2 * 128)
    mx = 0
    for ee in range(E):
        for jj in range(NB):
            mx = max(mx, int((sel[:, ee] & (tp == jj)).sum()))
    if mx > 128:
        raise ValueError(f"bucket overflow: {mx} > 128")
    return max(64, (mx + 7) // 8 * 8)


def prep_inputs(x, gate_w, wv1, w2, *_unused, BK=None):
    """Host-side shard/gather/cast/tile. Returns per-core input maps."""
    import ml_dtypes
    bf16 = ml_dtypes.bfloat16

    x = np.asarray(x, dtype=np.float32)
    gate_w = np.asarray(gate_w, dtype=np.float32)
    sel = _route_sel(x, gate_w)                       # [T, E] bool
    if BK is None:
        BK = fit_bk(sel)
    C = NB * BK
    tp = np.arange(T) // (2 * 128)                    # token-tile pair id

    xTf = np.ascontiguousarray(x.T).astype(np.float32)

    in_maps = []
    for c in range(NCORES):
        own = list(range(c * EPC, (c + 1) * EPC))
        rest = [i for i in range(E) if i not in own]
        perm = own + rest
        gp = gate_w[perm].T.astype(np.float32)        # [H, E]
        gwp = np.ascontiguousarray(
            gp.reshape(KH, 128, E).transpose(1, 0, 2).reshape(128, KH * E))

        toks = np.full((EPC, NB, BK), -1, dtype=np.int64)
        for le, ee in enumerate(own):
            for jj in range(NB):
                tt = np.nonzero(sel[:, ee] & (tp == jj))[0]
                if len(tt) > BK:
                    raise ValueError(
                        f"bucket overflow: expert {ee} pair {jj}: {len(tt)}")
                toks[le, jj, :len(tt)] = tt
        valid = toks >= 0
        tok0 = np.where(valid, toks, 0)

        xs = x[tok0.reshape(-1)].reshape(EPC, C, H) \
            * valid.reshape(EPC, C, 1)
        xgd = np.ascontiguousarray(
            xs.reshape(EPC, C, KH, 128).transpose(2, 3, 0, 1)
              .reshape(KH, 128, EPC * C)).astype(bf16)

        seldf = np.zeros((EPC, NB, BK, T), dtype=np.float32)
        il, ij, ii = np.nonzero(valid)
        seldf[il, ij, ii, toks[valid]] = 1.0
        seld = seldf.astype(bf16)

        wl = wv1[own]                                 # [EPC, 2F, H]
        wgd = np.ascontiguousarray(
            wl.reshape(EPC, MF2, 128, KH, 128)
              .transpose(0, 1, 4, 3, 2)
              .reshape(EPC, MF2, 128, KH * 128)).astype(bf16)

        w2l = w2[own]                                 # [EPC, H, F]
        w2d = np.ascontiguousarray(
            w2l.transpose(0, 2, 1).reshape(EPC, KF, 128, H)).astype(bf16)

        in_maps.append({
            "xT": xTf,
            "gwp": gwp,
            "xgd": xgd,
            "seld": seld,
            "wgd": wgd,
            "w2d": w2d,
        })
    return in_maps


def unshard(per_core_results):
    """Reassemble [T, H] from each core's stacked rs chunks."""
    shw = 2 * 128 // NCORES                           # 32 rows per chunk
    out = np.empty((T, H), dtype=np.float32)
    for c, res in enumerate(per_core_results):
        sh = np.asarray(res["out_sh"]).astype(np.float32)
        for jj in range(NB):
            base = jj * 2 * 128 + c * shw
            out[base:base + shw, :] = sh[jj * shw:(jj + 1) * shw, :]
    return out


def kernel(x, gate_w, wv1, w2, top_k):
    from concourse.bass_utils import run_bass_kernel_spmd

    assert int(top_k) == TOPK
    x = np.asarray(x, dtype=np.float32)
    gate_w = np.asarray(gate_w, dtype=np.float32)
    wv1 = np.asarray(wv1, dtype=np.float32)
    w2 = np.asarray(w2, dtype=np.float32)

    bk = fit_bk(_route_sel(x, gate_w))
    key = (T, H, F, E, NCORES, bk)
    if key not in _CACHE:
        _CACHE[key] = build_moe_nc(NCORES, BK=bk)
    nc = _CACHE[key]

    in_maps = prep_inputs(x, gate_w, wv1, w2, BK=bk)
    res = run_bass_kernel_spmd(nc, in_maps, list(range(NCORES)))
    return unshard([res.results[c] for c in range(NCORES)])


# revision 50
# speedup vs baseline: 1.0773x; 1.0773x over previous
"""Block-sparse MoE (sparse expert-parallel dispatch) Trainium2 kernel.

Problem: nn_BlockSparseMoE_15882789061249
  T=1024 tokens, H=2048 hidden, F=1408 intermediate, E=16 experts, top_k=6.

Strategy (8 NeuronCores, SPMD single program):
  - Expert parallel: core c owns experts {2c, 2c+1}; wv1/w2 sharded by
    expert on the host, gate replicated (columns permuted per core so the
    own experts land in route columns 0/1 -> one SPMD program).
  - Sparse dispatch: only top_k=6 of 16 experts contribute per token, so
    each expert needs only ~6/16 of the tokens. The host computes the
    routing *metadata* (which tokens each expert needs, with a 1e-4
    relative margin around the 6th prob so host/device top-k can never
    disagree) and ships per-expert gathered token matrices of capacity
    C=512 (actual max count is 418). All *numerics* stay on device: the
    fp32 router (logits -> softmax -> top-6 -> renorm), the expert MLPs
    on the gathered tokens, the route-weight combine, and the cross-core
    reduce-scatter.
  - Slots are bucketed by token-tile *pair* (4 buckets x 128 slots per
    expert; max actual bucket is 112), which makes the scatter-back
    pattern compile-time static: slot-chunk j only touches token tiles
    2j/2j+1. Scatter-back is a matmul with a host-provided 0/1 selection
    matrix, weighted on-device by the routed probabilities.
  - Weights are laid out so every DMA line is 2-4 KiB contiguous (the
    old per-[128,128]-tile layout moved 256 B lines and throttled the
    PE array to ~60% in phase A).
  - The reduce-scatter runs in 4 chunks of 2 token tiles, each fired as
    soon as its partial is complete, hiding most of the collective
    behind compute. Each core emits 4x [32, 2048] shards; the host
    reassembles them.
"""

import numpy as np

T, H, F, E = 1024, 2048, 1408, 16
NCORES = 8
TOPK = 6
EPC = E // NCORES            # experts per core (2)
NB = 4                       # slot buckets per expert (token-tile pairs)
KH = H // 128                # 16
KF = F // 128                # 11
MF2 = 2 * F // 128           # 22
MT = T // 128                # 8 token tiles
MARGIN = 1e-4                # relative margin on the 6th prob

_CACHE = {}


def build_moe_nc(n_cores=NCORES, BK=112):
    """Build + compile the SPMD Bass program for one core (same for all).

    BK = slot-bucket capacity (max tokens any expert draws from one
    token-tile pair, rounded up to 8). C = NB*BK is the per-expert
    gathered-token capacity; smaller BK means proportionally less
    phase-A matmul time, so it is fitted to the actual routing.
    """
    import concourse.bacc as bacc
    import concourse.mybir as mybir
    import concourse.tile as tile

    C = NB * BK

    f32 = mybir.dt.float32
    bf16 = mybir.dt.bfloat16
    AF = mybir.ActivationFunctionType
    Alu = mybir.AluOpType
    X = mybir.AxisListType.X

    t, e = T, E
    nc = bacc.Bacc("TRN2", target_bir_lowering=False, debug=False,
                   num_devices=n_cores)

    xT = nc.dram_tensor("xT", [H, t], f32, kind="ExternalInput")
    gwp = nc.dram_tensor("gwp", [128, KH * e], f32, kind="ExternalInput")
    xgd = nc.dram_tensor("xgd", [KH, 128, EPC * C], bf16,
                         kind="ExternalInput")
    seld = nc.dram_tensor("seld", [EPC, NB, BK, t], bf16,
                          kind="ExternalInput")
    wgd = nc.dram_tensor("wgd", [EPC, MF2, 128, KH * 128], bf16,
                         kind="ExternalInput")
    w2d = nc.dram_tensor("w2d", [EPC, KF, 128, H], bf16,
                         kind="ExternalInput")

    shw = 2 * 128 // n_cores
    parts = [nc.dram_tensor(f"partial{j}", [2 * 128, H], bf16)
             for j in range(NB)]
    rss = [nc.dram_tensor(f"rsi{j}", [shw, H], bf16) for j in range(NB)]
    out_sh = nc.dram_tensor("out_sh", [NB * shw, H], bf16,
                            kind="ExternalOutput")
    wrm_i = nc.dram_tensor("wrm_i", [8, 256], bf16)
    wrm_o = nc.dram_tensor("wrm_o", [1, 256], bf16)
    wrm2_i = nc.dram_tensor("wrm2_i", [8, 256], bf16)
    wrm2_o = nc.dram_tensor("wrm2_o", [1, 256], bf16)

    W2PRE = 8                # e0 w2 k-tiles prefetched before phase B

    with tile.TileContext(nc) as tc:
        with tc.tile_pool(name="persist", bufs=1) as pp:
            gw = pp.tile([128, KH * e], f32, tag="gw")
            lg = pp.tile([128, t], f32, tag="lg")
            route = pp.tile([128, MT * e + 32], f32, tag="route")
            ltr = pp.tile([128, MT * 32], f32, tag="ltr")
            rqs = pp.tile([128, EPC * t], f32, tag="rqs")
            rbc = pp.tile([128, EPC * t], f32, tag="rbc")
            act = pp.tile([128, EPC * KF * C], bf16, tag="act")
            sels = pp.tile([128, EPC * NB * t], bf16, tag="sels")
            selw = pp.tile([128, EPC * NB * t], bf16, tag="selw")
            w2p0 = pp.tile([128, W2PRE * H], bf16, tag="w2p0")
            ones = pp.tile([128, 128], f32, tag="ones")

            nc.sync.dma_start(out=gw[:], in_=gwp[:, :])
            nc.vector.memset(ones[0:32, :], 1.0)

            # rows 16:32 of lg feed the padded 32x32 transposes below; the
            # copy from psl overwrites rows :16 afterwards (32-aligned base)
            nc.vector.memset(lg[0:32, :], 0.0)
            nc.vector.memset(route[:, MT * e:], 0.0)
            nc.vector.memset(rqs[0:32, :], 0.0)

            # tiny collective up front absorbs the cold-start cost of the
            # CC path so the first real reduce-scatter runs at ring speed
            nc.gpsimd.collective_compute(
                "ReduceScatter", Alu.add,
                replica_groups=[list(range(n_cores))],
                ins=[wrm_i.ap().opt()],
                outs=[wrm_o.ap().opt()],
            )

            with (tc.tile_pool(name="xg", bufs=1) as pxg,
                  tc.tile_pool(name="wv", bufs=4) as pwv,
                  tc.tile_pool(name="xf", bufs=3) as pxf,
                  tc.tile_pool(name="sg", bufs=3) as psg,
                  tc.tile_pool(name="rt", bufs=2) as prt,
                  tc.tile_pool(name="psa", bufs=3, space="PSUM") as ppa):
                xg = pxg.tile([128, KH * EPC * C], bf16, tag="xg")

                def xg_dma(k):
                    nc.sync.dma_start(
                        out=xg[:, k * EPC * C:(k + 1) * EPC * C],
                        in_=xgd[k])

                # first few expert-pair weights and the xg tiles pair 0
                # consumes immediately go ahead of everything else in the
                # DMA queues so the PE can start within a few us
                NPRE = 3
                wpre = {}
                for mm in range(NPRE):
                    wg = pwv.tile([128, KH * 128], bf16, tag="wg",
                                  name=f"wgp{mm}")
                    nc.sync.dma_start(out=wg[:], in_=wgd[0, mm])
                    wu = pwv.tile([128, KH * 128], bf16, tag="wu",
                                  name=f"wup{mm}")
                    nc.sync.dma_start(out=wu[:], in_=wgd[0, KF + mm])
                    wpre[(0, mm)] = (wg, wu)
                    if mm == 0:
                        for k in range(4):
                            xg_dma(k)
                for k in range(4, KH):
                    xg_dma(k)

                def emit_a(le, mm, wgt, wut):
                    pg = ppa.tile([128, C], f32, tag="pg", name=f"pg{le}_{mm}")
                    pu = ppa.tile([128, C], f32, tag="pu", name=f"pu{le}_{mm}")
                    for k in range(KH):
                        rh = xg[:, k * EPC * C + le * C:
                                k * EPC * C + (le + 1) * C]
                        nc.tensor.matmul(pg[:],
                                         lhsT=wgt[:, k * 128:(k + 1) * 128],
                                         rhs=rh,
                                         start=(k == 0), stop=(k == KH - 1))
                    for k in range(KH):
                        rh = xg[:, k * EPC * C + le * C:
                                k * EPC * C + (le + 1) * C]
                        nc.tensor.matmul(pu[:],
                                         lhsT=wut[:, k * 128:(k + 1) * 128],
                                         rhs=rh,
                                         start=(k == 0), stop=(k == KH - 1))
                    sgm = psg.tile([128, C], bf16, tag="sgm",
                                   name=f"sgm{le}_{mm}")
                    nc.scalar.activation(sgm[:], pg[:], AF.Sigmoid)
                    sg = psg.tile([128, C], bf16, tag="sg",
                                  name=f"sg{le}_{mm}")
                    nc.vector.tensor_mul(out=sg[:], in0=sgm[:], in1=pg[:])
                    ai = (le * KF + mm) * C
                    nc.vector.tensor_mul(out=act[:, ai:ai + C],
                                         in0=sg[:], in1=pu[:])

                def emit_a_range(pairs):
                    for le, mm in pairs:
                        if (le, mm) in wpre:
                            emit_a(le, mm, *wpre[(le, mm)])
                            continue
                        wg = pwv.tile([128, KH * 128], bf16, tag="wg",
                                      name=f"wg{le}_{mm}")
                        nc.sync.dma_start(out=wg[:], in_=wgd[le, mm])
                        wu = pwv.tile([128, KH * 128], bf16, tag="wu",
                                      name=f"wu{le}_{mm}")
                        nc.sync.dma_start(out=wu[:], in_=wgd[le, KF + mm])
                        emit_a(le, mm, wg, wu)

                all_pairs = [(le, mm) for le in range(EPC)
                             for mm in range(KF)]
                # phase A, first slice: keeps the PE warm while the fp32
                # router matmuls (below) slot into the middle of the stream
                emit_a_range(all_pairs[:6])
                for le in range(EPC):
                    for jj in range(NB):
                        blk = (le * NB + jj) * t
                        nc.sync.dma_start(out=sels[0:BK, blk:blk + t],
                                          in_=seld[le, jj])
                emit_a_range(all_pairs[6:14])

                # ---- router logits [e, t] (fp32) ----
                with tc.tile_pool(name="psr", bufs=1,
                                  space="PSUM") as ppr:
                    psl = ppr.tile([128, t], f32, tag="psl")
                    for k in range(KH):
                        xf = pxf.tile([128, t], f32, tag="xf",
                                      name=f"xf{k}")
                        nc.sync.dma_start(out=xf[:],
                                          in_=xT[k * 128:(k + 1) * 128, :])
                        for n0 in range(0, t, 512):
                            nc.tensor.matmul(
                                psl[:e, n0:n0 + 512],
                                lhsT=gw[:, k * e:(k + 1) * e],
                                rhs=xf[:, n0:n0 + 512],
                                start=(k == 0), stop=(k == KH - 1))
                    nc.vector.tensor_copy(out=lg[:e, :], in_=psl[:e, :])

                # ---- top-k per token tile (all DVE; PE stays on MLPs) ----
                for tt in range(MT):
                    for b in range(4):
                        nc.vector.transpose(
                            out=ltr[b * 32:(b + 1) * 32,
                                    tt * 32:(tt + 1) * 32],
                            in_=lg[0:32, tt * 128 + b * 32:
                                   tt * 128 + (b + 1) * 32])
                    ev_in = ltr[:, tt * 32:tt * 32 + e]
                    mx = prt.tile([128, 1], f32, tag="mx")
                    nc.vector.reduce_max(out=mx[:], in_=ev_in, axis=X)
                    nm = prt.tile([128, 1], f32, tag="nm")
                    nc.vector.tensor_scalar_mul(nm[:], mx[:], -1.0)
                    ev = prt.tile([128, e], f32, tag="ev")
                    nc.scalar.activation(ev[:], ev_in, AF.Exp,
                                         bias=nm[:], scale=1.0)
                    t8 = prt.tile([128, 8], f32, tag="t8")
                    nc.vector.max(out=t8[:], in_=ev[:])
                    nc.vector.memset(t8[:, TOPK:], 0.0)
                    zap = prt.tile([128, e], f32, tag="zap")
                    nc.vector.match_replace(out=zap[:], in_to_replace=t8[:],
                                            in_values=ev[:], imm_value=0.0)
                    msk = prt.tile([128, e], f32, tag="msk")
                    nc.vector.tensor_sub(msk[:], ev[:], zap[:])
                    dn = prt.tile([128, 1], f32, tag="dn")
                    nc.vector.reduce_sum(out=dn[:], in_=msk[:], axis=X)
                    iv = prt.tile([128, 1], f32, tag="iv")
                    nc.vector.reciprocal(iv[:], dn[:])
                    nc.vector.tensor_scalar_mul(
                        route[:, tt * e:(tt + 1) * e], msk[:], iv[:])

                # route -> [expert, token] layout. The transpose input is
                # shifted by `le` so own-expert column le lands on partition
                # 0 of its block (compute APs need 32-aligned partition
                # bases, so reading rqs[1:2, :] later would be illegal).
                for le in range(EPC):
                    for tt in range(MT):
                        for b in range(4):
                            nc.vector.transpose(
                                out=rqs[0:32, le * t + tt * 128 + b * 32:
                                        le * t + tt * 128 + (b + 1) * 32],
                                in_=route[b * 32:(b + 1) * 32,
                                          tt * e + le:tt * e + le + 32])
                # ---- phase A, second slice, with the w2 prefetch DMAs
                # interleaved so they land just before phase B needs them
                emit_a_range(all_pairs[14:17])
                for kk in range(W2PRE):
                    nc.sync.dma_start(out=w2p0[:, kk * H:(kk + 1) * H],
                                      in_=w2d[0, kk])
                emit_a_range(all_pairs[17:])

                # broadcast each own-expert route row across all 128
                # partitions via a PE outer product (ones x row). Keeping
                # this off GpSimd matters: gpsimd ucode ops ahead of the
                # collectives were observed to stall the whole CC chain.
                with tc.tile_pool(name="psb2", bufs=1,
                                  space="PSUM") as ppb2:
                    for le in range(EPC):
                        pbc = ppb2.tile([128, t], f32, tag="pbc",
                                        name=f"pbc{le}")
                        for n0 in range(0, t, 512):
                            nc.tensor.matmul(
                                pbc[:, n0:n0 + 512],
                                lhsT=ones[0:1, :],
                                rhs=rqs[0:1, le * t + n0:le * t + n0 + 512],
                                start=True, stop=True)
                        nc.vector.tensor_copy(
                            out=rbc[:, le * t:(le + 1) * t], in_=pbc[:])
                for le in range(EPC):
                    for jj in range(NB):
                        blk = (le * NB + jj) * t
                        nc.vector.tensor_mul(
                            out=selw[0:BK, blk:blk + t],
                            in0=sels[0:BK, blk:blk + t],
                            in1=rbc[0:BK, le * t:(le + 1) * t])

            # second warm-up collective, data-gated on the end of phase A:
            # without it the CC path sits cold for ~160us and the first
            # real reduce-scatter ran 2-3x slower than the rest
            nc.sync.dma_start(out=wrm2_i[:, :],
                              in_=act[0:8, EPC * KF * C - 256:])
            nc.gpsimd.collective_compute(
                "ReduceScatter", Alu.add,
                replica_groups=[list(range(n_cores))],
                ins=[wrm2_i.ap().opt()],
                outs=[wrm2_o.ap().opt()],
            )

            # ---- phase B + weighted scatter + chunked reduce-scatter ----
            with (tc.tile_pool(name="w2p", bufs=EPC * KF - W2PRE) as pw2,
                  tc.tile_pool(name="yb", bufs=4) as pyb,
                  tc.tile_pool(name="so", bufs=2) as pso,
                  tc.tile_pool(name="psb", bufs=3, space="PSUM") as ppb,
                  tc.tile_pool(name="psc", bufs=2, space="PSUM") as ppc):
                w2sb = {}
                for le in range(EPC):
                    for kk in range(KF):
                        if le == 0 and kk < W2PRE:
                            w2sb[(le, kk)] = w2p0[:, kk * H:(kk + 1) * H]
                        else:
                            w2k = pw2.tile([128, H], bf16, tag="w2k")
                            nc.sync.dma_start(out=w2k[:], in_=w2d[le, kk])
                            w2sb[(le, kk)] = w2k[:]

                for jj in range(NB):
                    ybs = {}
                    for le in range(EPC):
                        py = [ppb.tile([128, 1024], f32, tag="py",
                                       name=f"py{jj}_{le}_{hh}")
                              for hh in range(2)]
                        for kk in range(KF):
                            lh = act[:, (le * KF + kk) * C + jj * BK:
                                     (le * KF + kk) * C + (jj + 1) * BK]
                            w2t_ = w2sb[(le, kk)]
                            for hh in range(2):
                                for q in range(2):
                                    n0 = q * 512
                                    nc.tensor.matmul(
                                        py[hh][0:BK, n0:n0 + 512],
                                        lhsT=lh,
                                        rhs=w2t_[:, hh * 1024 + n0:
                                                 hh * 1024 + n0 + 512],
                                        start=(kk == 0), stop=(kk == KF - 1))
                        yb = pyb.tile([128, H], bf16, tag="yb")
                        # ScalarE drains py so the DVE stays free for the
                        # scatter copies that gate the partial DMAs
                        for hh in range(2):
                            nc.scalar.activation(
                                yb[0:BK, hh * 1024:(hh + 1) * 1024],
                                py[hh][0:BK, :], AF.Copy)
                        ybs[le] = yb

                    for tt in (2 * jj, 2 * jj + 1):
                        so = pso.tile([128, H], bf16, tag="so",
                                      name=f"so{tt}")
                        for hq in range(4):
                            ps = ppc.tile([128, 512], f32, tag="ps",
                                          name=f"ps{tt}_{hq}")
                            for le in range(EPC):
                                blk = (le * NB + jj) * t
                                nc.tensor.matmul(
                                    ps[:],
                                    lhsT=selw[0:BK, blk + tt * 128:
                                              blk + (tt + 1) * 128],
                                    rhs=ybs[le][0:BK,
                                                hq * 512:(hq + 1) * 512],
                                    start=(le == 0), stop=(le == EPC - 1))
                            nc.vector.tensor_copy(
                                out=so[:, hq * 512:(hq + 1) * 512],
                                in_=ps[:])
                        # spread each partial store over 4 row-slice DMAs
                        # alternating between both DMA engines' ring sets:
                        # one ring moves ~40 GB/s, and backlogged partials
                        # delayed the reduce-scatter triggers 20-30 us
                        r0 = (tt % 2) * 128
                        for q in range(4):
                            eng = nc.sync if q % 2 == 0 else nc.scalar
                            eng.dma_start(
                                out=parts[jj][r0 + q * 32:
                                              r0 + (q + 1) * 32, :],
                                in_=so[q * 32:(q + 1) * 32, :])

                # collectives are emitted after the compute loop: each is
                # data-gated on its partial, so they still overlap B/scatter
                # of later chunks, but no sync edges land inside the PE/DVE
                # streams (in-loop emission measurably stalled both)
                for jj in range(NB):
                    nc.gpsimd.collective_compute(
                        "ReduceScatter", Alu.add,
                        replica_groups=[list(range(n_cores))],
                        ins=[parts[jj].ap().opt()],
                        outs=[rss[jj].ap().opt()],
                    )
                    nc.sync.dma_start(
                        out=out_sh[jj * shw:(jj + 1) * shw, :],
                        in_=rss[jj][:, :])

    nc.compile()
    return nc


def _route_sel(x, gate_w):
    """Host routing metadata: top-6 membership with a tie margin."""
    lg = x.astype(np.float64) @ gate_w.astype(np.float64).T
    lg -= lg.max(axis=1, keepdims=True)
    p = np.exp(lg)
    p /= p.sum(axis=1, keepdims=True)
    sp = -np.sort(-p, axis=1)
    thr = sp[:, TOPK - 1:TOPK] * (1.0 - MARGIN)
    return p >= thr


def fit_bk(sel):
    """Slot-bucket capacity: max (expert, tile-pair) count, rounded to 8."""
    tp = np.arange(T) // (2 * 128)
    mx = 0
    for ee in range(E):
        for jj in range(NB):
            mx = max(mx, int((sel[:, ee] & (tp == jj)).sum()))
    if mx > 128:
        raise ValueError(f"bucket overflow: {mx} > 128")
    return max(64, (mx + 7) // 8 * 8)


def prep_inputs(x, gate_w, wv1, w2, *_unused, BK=None):
    """Host-side shard/gather/cast/tile. Returns per-core input maps."""
    import ml_dtypes
    bf16 = ml_dtypes.bfloat16

    x = np.asarray(x, dtype=np.float32)
    gate_w = np.asarray(gate_w, dtype=np.float32)
    sel = _route_sel(x, gate_w)                       # [T, E] bool
    if BK is None:
        BK = fit_bk(sel)
    C = NB * BK
    tp = np.arange(T) // (2 * 128)                    # token-tile pair id

    xTf = np.ascontiguousarray(x.T).astype(np.float32)

    in_maps = []
    for c in range(NCORES):
        own = list(range(c * EPC, (c + 1) * EPC))
        rest = [i for i in range(E) if i not in own]
        perm = own + rest
        gp = gate_w[perm].T.astype(np.float32)        # [H, E]
        gwp = np.ascontiguousarray(
            gp.reshape(KH, 128, E).transpose(1, 0, 2).reshape(128, KH * E))

        toks = np.full((EPC, NB, BK), -1, dtype=np.int64)
        for le, ee in enumerate(own):
            for jj in range(NB):
                tt = np.nonzero(sel[:, ee] & (tp == jj))[0]
                if len(tt) > BK:
                    raise ValueError(
                        f"bucket overflow: expert {ee} pair {jj}: {len(tt)}")
                toks[le, jj, :len(tt)] = tt
        valid = toks >= 0
        tok0 = np.where(valid, toks, 0)

        xs = x[tok0.reshape(-1)].reshape(EPC, C, H) \
            * valid.reshape(EPC, C, 1)
        xgd = np.ascontiguousarray(
            xs.reshape(EPC, C, KH, 128).transpose(2, 3, 0, 1)
              .reshape(KH, 128, EPC * C)).astype(bf16)

        seldf = np.zeros((EPC, NB, BK, T), dtype=np.float32)
        il, ij, ii = np.nonzero(valid)
        seldf[il, ij, ii, toks[valid]] = 1.0
        seld = seldf.astype(bf16)

        wl = wv1[own]                                 # [EPC, 2F, H]
        wgd = np.ascontiguousarray(
            wl.reshape(EPC, MF2, 128, KH, 128)
              .transpose(0, 1, 4, 3, 2)
              .reshape(EPC, MF2, 128, KH * 128)).astype(bf16)

        w2l = w2[own]                                 # [EPC, H, F]
        w2d = np.ascontiguousarray(
            w2l.transpose(0, 2, 1).reshape(EPC, KF, 128, H)).astype(bf16)

        in_maps.append({
            "xT": xTf,
            "gwp": gwp,
            "xgd": xgd,
            "seld": seld,
            "wgd": wgd,
            "w2d": w2d,
        })
    return in_maps


def unshard(per_core_results):
    """Reassemble [T, H] from each core's stacked rs chunks."""
    shw = 2 * 128 // NCORES                           # 32 rows per chunk
    out = np.empty((T, H), dtype=np.float32)
    for c, res in enumerate(per_core_results):
        sh = np.asarray(res["out_sh"]).astype(np.float32)
        for jj in range(NB):
            base = jj * 2 * 128 + c * shw
            out[base:base + shw, :] = sh[jj * shw:(jj + 1) * shw, :]
    return out


def kernel(x, gate_w, wv1, w2, top_k):
    from concourse.bass_utils import run_bass_kernel_spmd

    assert int(top_k) == TOPK
    x = np.asarray(x, dtype=np.float32)
    gate_w = np.asarray(gate_w, dtype=np.float32)
    wv1 = np.asarray(wv1, dtype=np.float32)
    w2 = np.asarray(w2, dtype=np.float32)

    bk = fit_bk(_route_sel(x, gate_w))
    key = (T, H, F, E, NCORES, bk)
    if key not in _CACHE:
        _CACHE[key] = build_moe_nc(NCORES, BK=bk)
    nc = _CACHE[key]

    in_maps = prep_inputs(x, gate_w, wv1, w2, BK=bk)
    res = run_bass_kernel_spmd(nc, in_maps, list(range(NCORES)))
    return unshard([res.results[c] for c in range(NCORES)])
